# revision 17
# baseline (speedup 1.0000x reference)
"""MiniGPT forward on 8 Trainium2 NeuronCores — pure data parallel.

Model: V=8192, D=80, H=4 heads (hd=20), L=3 layers, T=64, B=512.
Sharding: batch 512 -> 64 sequences per core; outputs concatenated.

Per-core: 4096 tokens = 32 token-tiles of 128 (2 seqs each), as 16 pairs in
N_BLOCKS blocks. Residual h is token-major [128, 80] f32 in SBUF. Hot
matmuls run float32r (TF32-like; 1 cyc/row at free>=256). Head features are
padded to 32-partition boundaries (feature f of head h lives at partition
32h + f%20) because compute-engine access patterns must start at partition
0/32/64/96. Attention computes scoresT [k, 4hx128q] via a block-diagonal
padded q operand; softmax denominators via a ones-vector matmul +
reciprocal + gpsimd partition-broadcast; weights are normalized before the
attn-V matmul. LayerNorm scale/bias fold into adjacent weights on the host.

Block-major phases amortize ScalarE activation-table loads (sqrt/exp/gelu
live in different 1.28us-swap tables): per block+layer, sqrt ops batch,
then exps, then gelus.
"""

import sys

sys.path.insert(0, "/opt/trn_rl_repo")

import numpy as np
from contextlib import ExitStack

import concourse.bass as bass
import concourse.mybir as mybir
from concourse import bacc
from concourse.tile import TileContext
from concourse import bass_utils

F32 = mybir.dt.float32
F32R = mybir.dt.float32r
I32 = mybir.dt.int32
AF = mybir.ActivationFunctionType
OP = mybir.AluOpType

V, D, H, L, T = 8192, 80, 4, 3, 64
HD = D // H  # 20
EPS = 1e-5
N_CORES = 8
B = 512
B_LOC = B // N_CORES
N_TOK = B_LOC * T             # 4096
N_TILES = N_TOK // 128        # 32
N_PAIRS = N_TILES // 2        # 16
N_BLOCKS = 2
VCHUNK = 512
NVC = V // VCHUNK             # 16
LM_GRP = 2
SCALE = HD ** -0.5
NSLOT = 4
KLM = 97                      # lm lhsT rows: 80 feats + 16 zero + ones row at 96
AAP = 117                     # attn rows: padded 116 + ones row at 116

F1_CHUNKS = [(0, 128), (128, 256), (256, 320)]
PADPOS = [32 * (f // HD) + f % HD for f in range(D)]  # feature -> padded row


def f32(ap):
    return ap.bitcast(F32)


def build_nc(n_pairs=N_PAIRS, n_blocks=N_BLOCKS):
    nc = bacc.Bacc("TRN2", target_bir_lowering=False, debug=False)

    di = lambda name, shape, dt=F32: nc.dram_tensor(
        name, shape, dt, kind="ExternalInput"
    ).ap()
    x32 = di("x32", [128, N_TILES], I32)
    tok_emb = di("tok_emb", [V, D])
    tokT_aug = di("tokT_aug", [KLM, V])
    pos2 = di("pos2", [128, D])
    maskT = di("maskT", [128, H * 128])
    ident = di("ident", [128, 128])
    wqT = di("wqT", [L, D, 128])        # padded out-features
    wkT = di("wkT", [L, D, 128])
    wvT = di("wvT", [L, D, D])          # compact
    bqk = di("bqk", [L, 2, 128])        # padded
    bv2 = di("bv2", [L, 128, D])
    owT_aug = di("owT_aug", [L, AAP, D])
    w1T = di("w1T", [L, D, 4 * D])
    b1 = di("b1", [L, 4 * D])
    w2T_aug = di("w2T_aug", [L, 4 * D + 1, D])
    logits = nc.dram_tensor("logits", [N_TOK, V], F32, kind="ExternalOutput").ap()

    with TileContext(nc) as tc, ExitStack() as ctx:
        consts = ctx.enter_context(tc.tile_pool(name="consts", bufs=1))
        work = ctx.enter_context(tc.tile_pool(name="work", bufs=4))
        apool = ctx.enter_context(tc.tile_pool(name="apool", bufs=18))
        hpool = ctx.enter_context(tc.tile_pool(name="hpool", bufs=18))
        lmst = ctx.enter_context(tc.tile_pool(name="lmst", bufs=4))
        stat = ctx.enter_context(tc.tile_pool(name="stat", bufs=18))
        rdpool = ctx.enter_context(tc.tile_pool(name="rdpool", bufs=3))
        psA = ctx.enter_context(tc.tile_pool(name="psA", bufs=4, space="PSUM"))
        psM = ctx.enter_context(tc.tile_pool(name="psM", bufs=4, space="PSUM"))

        def load(dst_shape, src_ap, name, dt=F32):
            t = consts.tile(dst_shape, dt, tag=name, name=name)
            nc.sync.dma_start(out=t, in_=src_ap)
            return t

        def load_r(dst_shape, src_ap, name):
            s = consts.tile(dst_shape, F32, tag=name + "_s", name=name + "_s")
            nc.sync.dma_start(out=s, in_=src_ap)
            t = consts.tile(dst_shape, F32R, tag=name, name=name)
            nc.vector.tensor_copy(t, s)
            return t

        sb_x = load([128, N_TILES], x32, "sb_x", I32)
        sb_pos = load([128, D], pos2, "sb_pos")
        sb_mask = load([128, H * 128], maskT, "sb_mask")
        sb_id = load([128, 128], ident, "sb_id")
        # tokT is big (32KB/partition): load+round via a reused 8KB staging slot
        sb_tokT = consts.tile([KLM, V], F32R, tag="sb_tokT", name="sb_tokT")
        for ch in range(4):
            cs = slice(ch * (V // 4), (ch + 1) * (V // 4))
            tstg = consts.tile(
                [KLM, V // 4], F32, tag="tokT_stage", name="tokT_stage"
            )
            nc.sync.dma_start(out=tstg, in_=tokT_aug[:, cs])
            nc.vector.tensor_copy(sb_tokT[:, cs], tstg)

        sb_wq, sb_wk, sb_wv, sb_ow, sb_w1 = [], [], [], [], []
        sb_bq, sb_bk, sb_bv, sb_b1, sb_w2 = [], [], [], [], []
        for l in range(L):
            sb_wq.append(load_r([D, 128], wqT[l], f"wq{l}"))
            sb_wk.append(load_r([D, 128], wkT[l], f"wk{l}"))
            sb_wv.append(load([D, D], wvT[l], f"wv{l}"))
            sb_ow.append(load([AAP, D], owT_aug[l], f"ow{l}"))
            sb_w1.append(load_r([D, 4 * D], w1T[l], f"w1_{l}"))
            sb_bq.append(load([128, 1], bqk[l, 0][:, None], f"bq{l}"))
            sb_bk.append(load([128, 1], bqk[l, 1][:, None], f"bk{l}"))
            sb_bv.append(load([128, D], bv2[l], f"bv{l}"))
            sb_b1.append(
                [
                    load([e - s, 1], b1[l, s:e][:, None], f"b1_{l}_{ci}")
                    for ci, (s, e) in enumerate(F1_CHUNKS)
                ]
            )
            sb_w2.append(
                [
                    load(
                        [e - s + (1 if e == 4 * D else 0), D],
                        w2T_aug[l, s : e + (1 if e == 4 * D else 0)],
                        f"w2_{l}_{ci}",
                    )
                    for ci, (s, e) in enumerate(F1_CHUNKS)
                ]
            )

        eps_t = consts.tile([128, 1], F32)
        nc.vector.memset(eps_t, EPS)
        # walrus rejects memset on f32r APs: memset f32 sources, copy-to-round
        zeros_f = consts.tile([128, H * 128], F32)
        nc.vector.memset(zeros_f, 0.0)
        ones_f = consts.tile([128, 128], F32)
        nc.vector.memset(ones_f, 1.0)
        ones128 = consts.tile([128, 1], F32R)
        nc.vector.tensor_copy(ones128, ones_f[:, 0:1])

        q_bd, v_pad, attn_aug, g2_aug, hT_aug = [], [], [], [], []
        for i in range(NSLOT):
            qb = consts.tile([128, H * 128], F32R, tag=f"q_bd{i}")
            nc.vector.tensor_copy(qb, zeros_f)
            q_bd.append(qb)
            vp = consts.tile([128, 128], F32R, tag=f"v_pad{i}")
            nc.vector.tensor_copy(vp, zeros_f[:, :128])
            v_pad.append(vp)
            aa = consts.tile([AAP, 128], F32, tag=f"attn_aug{i}")
            nc.vector.memset(aa[0:96, :], 0.0)
            nc.vector.memset(aa[96:AAP, :], 1.0)  # row 116 stays the bias row
            attn_aug.append(aa)
            ga = consts.tile([65, 256], F32, tag=f"g2_aug{i}")
            nc.vector.memset(ga[64:65, :], 1.0)
            g2_aug.append(ga)
            ht = consts.tile([KLM, 128], F32R, tag=f"hT_aug{i}")
            nc.vector.tensor_copy(ht[64:KLM, :], zeros_f[64:KLM, :128])
            nc.vector.tensor_copy(ht[96:KLM, :], ones_f[96:KLM, :])
            hT_aug.append(ht)

        def ln_batch(h_list, out_tag):
            outs = []
            for h_t in h_list:
                st6 = stat.tile([128, 6], F32, tag="st6")
                nc.vector.bn_stats(st6, h_t)
                mv = stat.tile([128, 2], F32, tag="mv")
                nc.vector.bn_aggr(mv, st6)
                sd = stat.tile([128, 1], F32, tag="sd")
                nc.scalar.activation(sd, mv[:, 1:2], AF.Sqrt, bias=eps_t, scale=1.0)
                rstd = stat.tile([128, 1], F32, tag="rstd")
                nc.vector.reciprocal_approx_fast(out=rstd, in_=sd)
                nmr = stat.tile([128, 1], F32, tag="nmr")
                nc.vector.tensor_scalar(
                    out=nmr, in0=mv[:, 0:1], scalar1=rstd[:, 0:1], scalar2=-1.0,
                    op0=OP.mult, op1=OP.mult,
                )
                a_t = apool.tile([128, D], F32, tag=out_tag, name=out_tag)
                nc.scalar.activation(
                    a_t, h_t, AF.Identity, bias=nmr, scale=rstd[:, 0:1]
                )
                outs.append(a_t)
            return outs

        def transpose_pair(pair_tiles, tag):
            dstT = work.tile([D, 256], F32R, tag=tag, name=tag)
            for tl, src in enumerate(pair_tiles):
                ps = psM.tile([128, 256], F32, tag="med", name="ps_tr")[:D, :128]
                nc.tensor.transpose(ps, src, sb_id)
                nc.vector.tensor_copy(dstT[:, 128 * tl : 128 * (tl + 1)], ps)
            return dstT

        pairs_per_block = n_pairs // n_blocks
        assert pairs_per_block * n_blocks == n_pairs
        h_all = {}

        for blk in range(n_blocks):
            pair_ids = range(blk * pairs_per_block, (blk + 1) * pairs_per_block)
            tile_ids = [2 * p + tl for p in pair_ids for tl in range(2)]

            for t_idx in tile_ids:
                g = work.tile([128, D], F32, tag="gath")
                nc.gpsimd.indirect_dma_start(
                    out=g[:], out_offset=None, in_=tok_emb[:],
                    in_offset=bass.IndirectOffsetOnAxis(
                        ap=sb_x[:, t_idx : t_idx + 1], axis=0
                    ),
                )
                h_t = hpool.tile([128, D], F32, tag="h", name="h")
                nc.gpsimd.tensor_tensor(out=h_t, in0=g, in1=sb_pos, op=OP.add)
                h_all[t_idx] = h_t

            for l in range(L):
                # ======== attention phase ========
                a_all = ln_batch([h_all[t] for t in tile_ids], "a_t")
                for pi, p in enumerate(pair_ids):
                    a_pair = a_all[2 * pi : 2 * pi + 2]
                    aT2 = transpose_pair(a_pair, "aT2")

                    ps_q = psM.tile([128, 256], F32, tag="med", name="ps_q")
                    nc.tensor.matmul(
                        ps_q, lhsT=sb_wq[l], rhs=aT2, start=True, stop=True
                    )
                    ps_k = psM.tile([128, 256], F32, tag="med", name="ps_k")
                    nc.tensor.matmul(
                        ps_k, lhsT=sb_wk[l], rhs=aT2, start=True, stop=True
                    )
                    k_fm = work.tile([128, 256], F32R, tag="k_fm")
                    nc.scalar.activation(
                        k_fm, ps_k, AF.Identity, bias=sb_bk[l], scale=1.0
                    )
                    for tl in range(2):
                        qb = q_bd[(2 * pi + tl) % NSLOT]
                        tcols = slice(128 * tl, 128 * (tl + 1))
                        for hh in range(H):
                            rows = slice(32 * hh, 32 * hh + HD)
                            if hh % 2 == 0:
                                nc.vector.tensor_scalar_add(
                                    out=qb[rows, 128 * hh : 128 * (hh + 1)],
                                    in0=ps_q[rows, tcols],
                                    scalar1=sb_bq[l][rows, 0:1],
                                )
                            else:
                                nc.scalar.activation(
                                    qb[rows, 128 * hh : 128 * (hh + 1)],
                                    ps_q[rows, tcols],
                                    AF.Identity,
                                    bias=sb_bq[l][rows, 0:1],
                                    scale=1.0,
                                )
                    for tl in range(2):
                        tcols = slice(128 * tl, 128 * (tl + 1))
                        ps_v = psM.tile([128, 256], F32, tag="med", name="ps_v")[
                            :, :D
                        ]
                        nc.tensor.matmul(
                            ps_v, lhsT=f32(aT2[:, tcols]), rhs=sb_wv[l],
                            start=True, stop=True,
                        )
                        vp = v_pad[(2 * pi + tl) % NSLOT]
                        vview = vp.rearrange("k (h c) -> k h c", c=32)[:, :, :HD]
                        nc.vector.tensor_tensor(
                            out=vview,
                            in0=ps_v.rearrange("k (h c) -> k h c", c=HD),
                            in1=sb_bv[l].rearrange("k (h c) -> k h c", c=HD),
                            op=OP.add,
                        )

                    for tl in range(2):
                        slot = (2 * pi + tl) % NSLOT
                        h_t = h_all[2 * p + tl]
                        tcols = slice(128 * tl, 128 * (tl + 1))
                        ps_s = psA.tile([128, H * 128], F32, tag="big", name="ps_s")
                        nc.tensor.matmul(
                            ps_s, lhsT=k_fm[:, tcols], rhs=q_bd[slot],
                            start=True, stop=True,
                        )
                        w_sb = work.tile([128, H * 128], F32R, tag="w_sb")
                        nc.scalar.activation(w_sb, ps_s, AF.Exp, scale=SCALE)
                        nc.vector.tensor_tensor(
                            out=w_sb, in0=f32(w_sb), in1=sb_mask, op=OP.mult
                        )
                        ps_den = psM.tile(
                            [1, H * 128], F32, tag="med", name="ps_den"
                        )
                        nc.tensor.matmul(
                            ps_den, lhsT=ones128, rhs=w_sb, start=True, stop=True
                        )
                        rd = rdpool.tile([1, H * 128], F32, tag="rd")
                        nc.vector.reciprocal_approx_fast(out=rd, in_=ps_den)
                        rdb = work.tile([128, H * 128], F32, tag="rdb")
                        nc.gpsimd.partition_broadcast(rdb, rd)
                        nc.vector.tensor_tensor(
                            out=w_sb, in0=f32(w_sb), in1=rdb, op=OP.mult
                        )
                        ps_at = psA.tile([128, H * 128], F32, tag="big", name="ps_at")
                        nc.tensor.matmul(
                            ps_at, lhsT=v_pad[slot], rhs=w_sb, start=True, stop=True
                        )
                        aa = attn_aug[slot]
                        for hh in range(H):
                            rows = slice(32 * hh, 32 * hh + HD)
                            nc.scalar.activation(
                                aa[rows, :],
                                ps_at[rows, 128 * hh : 128 * (hh + 1)],
                                AF.Copy,
                            )
                        ps_o = psM.tile([128, 256], F32, tag="med", name="ps_o")[
                            :, :D
                        ]
                        nc.tensor.matmul(
                            ps_o, lhsT=aa, rhs=sb_ow[l], start=True, stop=True
                        )
                        nc.vector.tensor_tensor(
                            out=h_t, in0=h_t, in1=ps_o, op=OP.add
                        )

                # ======== ffn phase ========
                f_all = ln_batch([h_all[t] for t in tile_ids], "f_t")
                for pi, p in enumerate(pair_ids):
                    fT2 = transpose_pair(f_all[2 * pi : 2 * pi + 2], "fT2")
                    g_chunks = []
                    for ci, (s, e) in enumerate(F1_CHUNKS):
                        m = e - s
                        ps_f = psA.tile([m, 256], F32, tag="big", name="ps_f")
                        nc.tensor.matmul(
                            ps_f, lhsT=sb_w1[l][:, s:e], rhs=fT2,
                            start=True, stop=True,
                        )
                        if ci == 2:
                            g_c = g2_aug[pi % NSLOT]
                            nc.scalar.activation(
                                g_c[:64, :], ps_f, AF.Gelu,
                                bias=sb_b1[l][ci], scale=1.0,
                            )
                        else:
                            g_c = work.tile([m, 256], F32, tag=f"g{ci}")
                            nc.scalar.activation(
                                g_c, ps_f, AF.Gelu, bias=sb_b1[l][ci], scale=1.0
                            )
                        g_chunks.append(g_c)
                    for tl in range(2):
                        tcols = slice(128 * tl, 128 * (tl + 1))
                        ps_2 = psM.tile([128, 256], F32, tag="med", name="ps_2")[
                            :, :D
                        ]
                        for ci in range(3):
                            nc.tensor.matmul(
                                ps_2, lhsT=g_chunks[ci][:, tcols],
                                rhs=sb_w2[l][ci],
                                start=(ci == 0), stop=(ci == 2),
                            )
                        h_t = h_all[2 * p + tl]
                        nc.vector.tensor_tensor(
                            out=h_t, in0=h_t, in1=ps_2, op=OP.add
                        )

            # ======== lm head phase ========
            z_all = ln_batch([h_all[t] for t in tile_ids], "z_t")
            for ti, t_idx in enumerate(tile_ids):
                ps_tr = psM.tile([128, 256], F32, tag="med", name="ps_tr2")[
                    :D, :128
                ]
                nc.tensor.transpose(ps_tr, z_all[ti], sb_id)
                ht = hT_aug[ti % NSLOT]
                nc.vector.tensor_copy(ht[:D, :], ps_tr)
                for j in range(NVC // LM_GRP):
                    stg = lmst.tile([128, LM_GRP * VCHUNK], F32, tag="stg")
                    for k2 in range(LM_GRP):
                        c = LM_GRP * j + k2
                        ps_lm = psA.tile(
                            [128, VCHUNK], F32, tag="big", name="ps_lm"
                        )
                        nc.tensor.matmul(
                            ps_lm, lhsT=ht,
                            rhs=sb_tokT[:, VCHUNK * c : VCHUNK * (c + 1)],
                            start=True, stop=True,
                        )
                        dst = stg[:, VCHUNK * k2 : VCHUNK * (k2 + 1)]
                        if c % 8 < 5:
                            nc.scalar.activation(dst, ps_lm, AF.Copy)
                        else:
                            nc.vector.tensor_copy(dst, ps_lm)
                    nc.sync.dma_start(
                        out=logits[
                            128 * t_idx : 128 * (t_idx + 1),
                            LM_GRP * VCHUNK * j : LM_GRP * VCHUNK * (j + 1),
                        ],
                        in_=stg,
                    )

    nc.compile()
    return nc


# ------------------------- host side -------------------------

def _np(a, dt=np.float32):
    return np.ascontiguousarray(np.asarray(a), dtype=dt)


def prepare_shared(tok_emb, pos_emb, ln1_s, ln1_b, ln2_s, ln2_b,
                   qkv_w, qkv_b, out_w, out_b, ffn_w1, ffn_b1, ffn_w2, ffn_b2,
                   lnf_s, lnf_b):
    """Fold LN scale/bias into adjacent weights; build device input arrays."""
    tok_emb = _np(tok_emb); pos_emb = _np(pos_emb)
    ln1_s = _np(ln1_s); ln1_b = _np(ln1_b)
    ln2_s = _np(ln2_s); ln2_b = _np(ln2_b)
    qkv_w = _np(qkv_w); qkv_b = _np(qkv_b)
    out_w = _np(out_w); out_b = _np(out_b)
    ffn_w1 = _np(ffn_w1); ffn_b1 = _np(ffn_b1)
    ffn_w2 = _np(ffn_w2); ffn_b2 = _np(ffn_b2)
    lnf_s = _np(lnf_s); lnf_b = _np(lnf_b)
    pp = np.asarray(PADPOS)

    d = {}
    d["tok_emb"] = tok_emb
    tokT_aug = np.zeros((KLM, V), np.float32)
    tokT_aug[:D] = (tok_emb * lnf_s[None, :]).T
    tokT_aug[96] = lnf_b @ tok_emb.T
    d["tokT_aug"] = tokT_aug
    d["pos2"] = np.concatenate([pos_emb, pos_emb], axis=0)

    k = np.arange(128)[:, None]
    q = np.arange(128)[None, :]
    m = ((k // T == q // T) & (k % T <= q % T)).astype(np.float32)
    d["maskT"] = np.tile(m, (1, H))
    d["ident"] = np.eye(128, dtype=np.float32)

    wq = qkv_w[:, 0:D, :]; wk = qkv_w[:, D:2 * D, :]; wv = qkv_w[:, 2 * D:, :]
    wqT = np.zeros((L, D, 128), np.float32)
    wkT = np.zeros((L, D, 128), np.float32)
    for l in range(L):
        wqT[l][:, pp] = (wq[l] * ln1_s[l][None, :]).T
        wkT[l][:, pp] = (wk[l] * ln1_s[l][None, :]).T
    d["wqT"] = wqT
    d["wkT"] = wkT
    d["wvT"] = np.stack([(wv[l] * ln1_s[l][None, :]).T for l in range(L)])
    bqk = np.zeros((L, 2, 128), np.float32)
    for l in range(L):
        bqk[l, 0][pp] = qkv_b[l, 0:D] + wq[l] @ ln1_b[l]
        bqk[l, 1][pp] = qkv_b[l, D:2 * D] + wk[l] @ ln1_b[l]
    d["bqk"] = bqk
    bv = np.stack([qkv_b[l, 2 * D:] + wv[l] @ ln1_b[l] for l in range(L)])
    d["bv2"] = np.broadcast_to(bv[:, None, :], (L, 128, D)).copy()
    ow = np.zeros((L, AAP, D), np.float32)
    for l in range(L):
        ow[l][pp] = out_w[l].T
        ow[l, AAP - 1] = out_b[l]
    d["owT_aug"] = ow
    d["w1T"] = np.stack([(ffn_w1[l] * ln2_s[l][None, :]).T for l in range(L)])
    d["b1"] = np.stack([ffn_b1[l] + ffn_w1[l] @ ln2_b[l] for l in range(L)])
    w2 = np.empty((L, 4 * D + 1, D), np.float32)
    for l in range(L):
        w2[l, : 4 * D] = ffn_w2[l].T
        w2[l, 4 * D] = ffn_b2[l]
    d["w2T_aug"] = w2
    return d


_NC_CACHE = {}


def get_nc(n_pairs=N_PAIRS, n_blocks=N_BLOCKS):
    key = (n_pairs, n_blocks)
    if key not in _NC_CACHE:
        _NC_CACHE[key] = build_nc(n_pairs, n_blocks)
    return _NC_CACHE[key]


def kernel(x, tok_emb, pos_emb, ln1_s, ln1_b, ln2_s, ln2_b,
           qkv_w, qkv_b, out_w, out_b, ffn_w1, ffn_b1, ffn_w2, ffn_b2,
           lnf_s, lnf_b, _trace=False, _n_cores=N_CORES):
    x = np.ascontiguousarray(np.asarray(x)).astype(np.int64, copy=False)
    shared = prepare_shared(
        tok_emb, pos_emb, ln1_s, ln1_b, ln2_s, ln2_b, qkv_w, qkv_b,
        out_w, out_b, ffn_w1, ffn_b1, ffn_w2, ffn_b2, lnf_s, lnf_b
    )
    nc = get_nc()
    in_maps = []
    for c in range(_n_cores):
        xs = x[c * B_LOC : (c + 1) * B_LOC].reshape(-1).astype(np.int32)
        m = dict(shared)
        m["x32"] = np.ascontiguousarray(xs.reshape(N_TILES, 128).T)
        in_maps.append(m)
    res = bass_utils.run_bass_kernel_spmd(
        nc, in_maps, list(range(_n_cores)), trace=_trace
    )
    outs = [res.results[c]["logits"].reshape(B_LOC, T, V) for c in range(_n_cores)]
    full = np.concatenate(outs, axis=0)
    if _trace:
        return full, res
    return full
